# revision 21
# baseline (speedup 1.0000x reference)
"""Self-contained Trainium2 Bass kernel: pre-LN multi-head attention block.

Computes, for x [B=8, S=1024, D=1024] (fp32) and packed attention weights:
    out = x + out_proj(MHA(LayerNorm(x)))
matching torch nn.MultiheadAttention's explicit (non-flash) path with 16 heads.

Sharding: data-parallel over batch - core i handles batch element i; no
collectives, outputs are concatenated on the host.

Per-core strategy (v2):
  - x arrives twice: bf16 x^T for LN stats, fp8 x^T (plane layout) for the
    QKV DoubleRow matmuls. LayerNorm is folded ALGEBRAICALLY into QKV:
      q = rstd[s]*(x@W) - mu[s]*colsum(W) (+ gamma pre-folded into W),
    so the normalized activations are never materialized. The rank-1
    mu*w1 correction is one extra 1-row matmul pass per output tile; the
    rstd multiply rides the mandatory PSUM->SBUF drain.
  - LN stats are 1-column-moving matmuls (cost ~1 cycle each in the PE
    cost model), chained through a PE transpose and broadcast DMAs.
  - q/k biases (LN beta fold) are folded into the softmax: the bq.K term
    becomes the exp's per-partition bias (tiny 1-col matmuls); the bk.q
    term is constant per softmax column and cancels between num/denom.
  - softmax exp splits across TWO engines: most tiles on ACT
    (Exp, scale=2^-13), the rest via a DVE psum->sbuf copy + gpsimd
    tensor_tensor pow(e^(2^-13), scores) - the Pool engine computes exp
    at 0.6 efficiency but runs fully parallel to ACT.
  - PV uses a wide [v | ones/16] stationary so the softmax denominator
    accumulates in psum partitions 64-127 of the same matmul (denominator
    matmuls cost nothing extra); ctx is normalized at x16 scale to stay
    in fp8-normal range, with the v-bias folded via ctx = PV/den + bv.
  - out-proj is split in two head-halves so the final drain+DMA overlaps
    the last heads' attention.
"""

import numpy as np
import ml_dtypes

P = 128
D = 1024
H = 16
DH = 64
B = 8
S = 1024
LN_EPS = 1e-5
N_CORES = 8

_ND = D // P   # d tiles (8)
NS = S // P    # s tiles (8)
WS = 32.0      # fp8 weight pre-scale (power of two)
EXP_SCALE = 0.125 / 1024.0      # 2^-13: logits = psum * 2^-13
CS = 16.0                       # ctx fp8 scale (den ones = 1/CS)
OUT_SCALE = 1.0 / (CS * WS)     # 1/512
EBASE = float(np.exp(2.0 ** -13))
EXP_BIAS = -3.0   # cancels in softmax; keeps fp8 exp finite

# exp tiles routed to the Pool engine: (tt indices per (hp, idx) head)
POOL_TT = (7,)

LAST_RESULTS = None
_NC_CACHE = {}


def _emit(tc, aps):
    from concourse import mybir
    from concourse.masks import make_identity

    nc = tc.nc
    f32 = mybir.dt.float32
    bf16 = mybir.dt.bfloat16
    fp8 = mybir.dt.float8e4
    FT = mybir.ActivationFunctionType
    OP = mybir.AluOpType
    DR = mybir.MatmulPerfMode.DoubleRow

    xT, x8d, resid, wqkt, wvt, woutt, negw1qk, negw1v, bq32, binv, out = (
        aps["xt"], aps["x8"], aps["resid"], aps["wqkt"], aps["wvt"],
        aps["woutt"], aps["negw1qk"], aps["negw1v"], aps["bq32"],
        aps["binv"], aps["out"],
    )
    dbg_qkT, dbg_bbc, dbg_ex, dbg_ch = aps["dbg_qkT"], aps["dbg_bbc"], aps["dbg_ex"], aps["dbg_ch"]

    with tc.tile_pool(name="consts", bufs=1) as consts, \
         tc.tile_pool(name="acts", bufs=1) as acts, \
         tc.tile_pool(name="wpool", bufs=1) as wpool:

        # ---------- constants ----------
        ident = consts.tile([P, P], bf16)
        make_identity(nc, ident[:])
        ones_mat = consts.tile([P, P], bf16)
        nc.vector.memset(ones_mat, 1.0)
        ones1 = consts.tile([P, 1], bf16)
        nc.vector.memset(ones1, 1.0)
        # row-selector stationaries: sel[p, r, f] = (p == r)
        sel = consts.tile([16, 16, P], bf16)
        nc.gpsimd.memset(sel, 0.0)
        nc.gpsimd.affine_select(
            out=sel, in_=sel, compare_op=mybir.AluOpType.not_equal,
            fill=1.0, base=0,
            # keep 0 where (p - r) != 0, fill 1.0 on p == r
            pattern=[[-1, 16], [0, P]], channel_multiplier=1)
        eps_sb = consts.tile([P, 1], f32)
        nc.vector.memset(eps_sb, LN_EPS)
        ebase = consts.tile([P, S], f32)
        nc.vector.memset(ebase, EBASE)
        binv_bc = consts.tile([P, D], f32)
        nw1qk_sb = consts.tile([1, 2 * D], bf16)
        nw1v_sb = consts.tile([1, D], bf16)
        bq32_sb = consts.tile([P, H], fp8)

        # ---------- persistent activations ----------
        xT_sb = acts.tile([P, _ND, S], bf16)     # x^T bf16 (LN stats)
        x8_sb = acts.tile([P, _ND, S], fp8)      # x^T fp8 (QKV moving/stat)
        qkT = acts.tile([P, 2 * _ND, S], fp8)    # q tiles 0..7, k tiles 8..15
        # V wide stationary: [t%128, ttpair, plane, h, 64 v | 64 ones/CS]
        v8w = acts.tile([P, NS // 2, 2, H, P], fp8)
        ctxT8 = acts.tile([P, _ND, S], fp8)      # x16-scaled normalized ctx^T
        resid_sb = acts.tile([P, NS, D], bf16)   # x + out_proj_b, natural
        stage_sb = acts.tile([P, NS, D], bf16)   # heads 0-7 partial + resid
        b_bc = acts.tile([P, S], bf16)           # rstd, partition-replicated
        mu_bc = acts.tile([P, S], bf16)          # mu, partition-replicated
        bK_sb = acts.tile([P, H, NS], f32)       # exp bias: 2^-13*(bq.K)[t]
        bKr_sb = acts.tile([P, H, NS], f32)      # same, raw psum scale
        b32_sb = acts.tile([P, NS], f32)         # rstd/WS per t (V drain)

        # ---------- weights (fp8, pre-scaled by WS on host) ----------
        wqk_sb = wpool.tile([P, _ND, 2 * D], fp8)
        wv_sb = wpool.tile([P, _ND, D], fp8)
        wout_sb = wpool.tile([P, _ND, D], fp8)

        # memset the ones-planes of the V wide stationary (den columns)
        nc.gpsimd.memset(v8w[:, :, :, :, DH:P], 1.0 / CS)

        # ================= Phase 1: LayerNorm stats =================
        with tc.tile_pool(name="lnps", bufs=1, space="PSUM") as lnps, \
             tc.tile_pool(name="lntmp", bufs=1) as lntmp:
            stats_ps = lnps.tile([P, 2 * NS], f32, tag="stats")
            tps = lnps.tile([16, P], bf16, tag="tps")
            rep_ps = lnps.tile([P, 2 * S], f32, tag="rep")
            warm_ps = lnps.tile([P, P], f32, tag="warm")
            for _ in range(24):
                nc.tensor.matmul(warm_ps, lhsT=ones_mat, rhs=ones_mat,
                                 start=True, stop=True)
            xT_r = xT.rearrange("(a p) s -> p a s", p=P)
            NCK = 512
            for c in range(2):
                sl = slice(c * NCK, (c + 1) * NCK)
                for j in range(_ND):
                    nc.sync.dma_start(out=xT_sb[:, j, sl], in_=xT_r[:, j, sl])
            nc.sync.dma_start(out=x8_sb,
                              in_=x8d.rearrange("(a p) s -> p a s", p=P))
            wqkt_r2 = wqkt.rearrange("(a p) e -> p a e", p=P)
            nc.sync.dma_start(out=wqk_sb[:, :, 0:2 * P],
                              in_=wqkt_r2[:, :, 0:2 * P])
            nc.sync.dma_start(out=wqk_sb[:, :, 8 * P:10 * P],
                              in_=wqkt_r2[:, :, 8 * P:10 * P])
            nc.sync.dma_start(out=wqk_sb[:, :, 2 * P:8 * P],
                              in_=wqkt_r2[:, :, 2 * P:8 * P])
            nc.sync.dma_start(out=wqk_sb[:, :, 10 * P:16 * P],
                              in_=wqkt_r2[:, :, 10 * P:16 * P])
            nc.sync.dma_start(out=wv_sb,
                              in_=wvt.rearrange("(a p) e -> p a e", p=P))
            nc.sync.dma_start(out=wout_sb,
                              in_=woutt.rearrange("(a p) e -> p a e", p=P))
            nc.sync.dma_start(out=resid_sb,
                              in_=resid.rearrange("(st p) e -> p st e", p=P))
            nc.gpsimd.dma_start(out=binv_bc,
                                in_=binv[None, :].to_broadcast((P, D)))
            nc.gpsimd.dma_start(out=nw1qk_sb, in_=negw1qk)
            nc.gpsimd.dma_start(out=nw1v_sb, in_=negw1v)
            nc.gpsimd.dma_start(out=bq32_sb, in_=bq32)

            # squares (DVE, bf16 2x) + 1-col stats matmuls; accumulation
            # groups must be strictly sequential (interleaved start/stop
            # groups in one psum bank corrupt each other)
            for c in range(2):
                sqs = []
                with nc.allow_low_precision(reason="x^2 for LN stats"):
                    for j in range(_ND):
                        sl = slice(c * NCK, (c + 1) * NCK)
                        sq = lntmp.tile([P, NCK], bf16, tag="sq", bufs=16,
                                        name=f"sq{c}_{j}")
                        nc.vector.tensor_tensor(out=sq, in0=xT_sb[:, j, sl],
                                                in1=xT_sb[:, j, sl],
                                                op=OP.mult)
                        sqs.append(sq)
                for st in range(c * 4, c * 4 + 4):
                    o = st * P - c * NCK
                    for j in range(_ND):
                        nc.tensor.matmul(stats_ps[:, st:st + 1],
                                         lhsT=xT_sb[:, j, st * P:(st + 1) * P],
                                         rhs=ones1,
                                         start=(j == 0), stop=(j == _ND - 1))
                    for j in range(_ND):
                        nc.tensor.matmul(stats_ps[:, NS + st:NS + st + 1],
                                         lhsT=sqs[j][:, o:o + P], rhs=ones1,
                                         start=(j == 0), stop=(j == _ND - 1))

            # chain on [128, 8] tiles: partition = s%128, col = s-tile
            ch = lntmp.tile([P, 6 * NS], f32, tag="ch")
            mu = ch[:, 0:NS]
            var = ch[:, NS:2 * NS]
            std = ch[:, 2 * NS:3 * NS]
            b8f = ch[:, 3 * NS:4 * NS]
            nc.vector.tensor_scalar_mul(mu, stats_ps[:, 0:NS], 1.0 / D)
            with nc.allow_low_precision(reason="LN chain"):
                musq = ch[:, 4 * NS:5 * NS]
                nc.vector.tensor_tensor(out=musq, in0=mu, in1=mu, op=OP.mult)
                nc.vector.scalar_tensor_tensor(
                    out=var, in0=stats_ps[:, NS:2 * NS], scalar=1.0 / D,
                    in1=musq, op0=OP.mult, op1=OP.subtract)
            nc.scalar.activation(out=std, in_=var, func=FT.Sqrt, bias=eps_sb)
            nc.vector.reciprocal(out=b8f, in_=std)
            nc.vector.tensor_scalar_mul(b32_sb, b8f, 1.0 / WS)
            bmu = lntmp.tile([P, 16], bf16, tag="bmu")
            with nc.allow_low_precision(reason="LN stats to bf16"):
                nc.vector.tensor_copy(out=bmu[:, 0:NS], in_=b8f)
                nc.vector.tensor_copy(out=bmu[:, NS:16], in_=mu)
            nc.sync.dma_start(out=dbg_ch, in_=ch)
            nc.tensor.transpose(out=tps, in_=bmu, identity=ident)
            tsb = lntmp.tile([16, P], bf16, tag="tsb")
            nc.vector.tensor_copy(out=tsb, in_=tps)
            # replicate b and mu across partitions via selector matmuls
            for st in range(NS):
                nc.tensor.matmul(rep_ps[:, st * P:(st + 1) * P],
                                 lhsT=sel[:, st, :], rhs=tsb,
                                 start=True, stop=True)
                nc.tensor.matmul(rep_ps[:, S + st * P:S + (st + 1) * P],
                                 lhsT=sel[:, NS + st, :], rhs=tsb,
                                 start=True, stop=True)
            with nc.allow_low_precision(reason="LN bcast to bf16"):
                nc.vector.tensor_copy(out=b_bc, in_=rep_ps[:, 0:S])
                nc.vector.tensor_copy(out=mu_bc, in_=rep_ps[:, S:2 * S])

        # ============ Phases 2-4: projections + attention + out-proj ========
        with tc.tile_pool(name="expool", bufs=1) as expool, \
             tc.tile_pool(name="sidep", bufs=1) as sidep, \
             tc.tile_pool(name="mps", bufs=1, space="PSUM") as mps:

            def dr(ps_out, lhsT, rhs, start, stop):
                nc.tensor.matmul(ps_out, lhsT=lhsT, rhs=rhs, start=start,
                                 stop=stop, perf_mode=DR)

            def emit_qk_unit(et):
                # one q|k e-tile s-half [128e, 512]: 4 DR + mu*w1 correction
                e0 = et * P
                for half in range(2):
                    ps = mps.tile([P, 512], f32, tag="mm", bufs=3,
                                  name=f"qk{et}_{half}")
                    sl = slice(half * 512, (half + 1) * 512)
                    for c2 in range(2):
                        s2 = slice(half * 512 + c2 * 256,
                                   half * 512 + (c2 + 1) * 256)
                        for jp in range(_ND // 2):
                            dr(ps[:, c2 * 256:(c2 + 1) * 256],
                               wqk_sb[:, 2 * jp:2 * jp + 2, e0:e0 + P],
                               x8_sb[:, 2 * jp:2 * jp + 2, s2],
                               start=(jp == 0), stop=False)
                        nc.tensor.matmul(ps[:, c2 * 256:(c2 + 1) * 256],
                                         lhsT=nw1qk_sb[0:1, e0:e0 + P],
                                         rhs=mu_bc[0:1, s2],
                                         start=False, stop=True)
                    with nc.allow_low_precision(reason="qk to bf16"):
                        nc.vector.tensor_tensor(out=qkT[:, et, sl], in0=ps,
                                                in1=b_bc[:, sl], op=OP.mult)

            def emit_bk(et):
                # exp-bias fold: bK[t, h] = 2^-13 * sum_dh bq[dh] k[dh, t]
                # (k e-tile et covers heads 2(et-8), 2(et-8)+1)
                bps = mps.tile([P, 2 * NS], f32, tag="bk", bufs=1,
                               name=f"bk{et}")
                for idx in range(2):
                    h = 2 * (et - 8) + idx
                    base = idx * DH
                    for tt in range(NS):
                        nc.tensor.matmul(
                            bps[:, idx * NS + tt:idx * NS + tt + 1],
                            lhsT=qkT[base:base + DH, et, tt * P:(tt + 1) * P],
                            rhs=bq32_sb[base:base + DH, h:h + 1],
                            start=True, stop=True, tile_position=(base, 0))
                for idx in range(2):
                    h = 2 * (et - 8) + idx
                    sl = slice(idx * NS, (idx + 1) * NS)
                    nc.vector.tensor_scalar(out=bK_sb[:, h, :], in0=bps[:, sl],
                                            scalar1=EXP_SCALE, scalar2=EXP_BIAS,
                                            op0=OP.mult, op1=OP.add)
                    nc.vector.tensor_scalar(out=bKr_sb[:, h, :], in0=bps[:, sl],
                                            scalar1=1.0,
                                            scalar2=EXP_BIAS * 8192.0,
                                            op0=OP.mult, op1=OP.add)

            def emit_v_unit(st):
                # V natural [t-tile, e'=(h,dh)] per e'-half: 4 DR + mu*w1v fix
                t0 = st * P
                for eh in range(2):
                    ps = mps.tile([P, 512], f32, tag="mm", bufs=3,
                                  name=f"v{st}_{eh}")
                    sl = slice(eh * 512, (eh + 1) * 512)
                    for c2 in range(2):
                        s2 = slice(eh * 512 + c2 * 256,
                                   eh * 512 + (c2 + 1) * 256)
                        for jp in range(_ND // 2):
                            dr(ps[:, c2 * 256:(c2 + 1) * 256],
                               x8_sb[:, 2 * jp:2 * jp + 2, t0:t0 + P],
                               wv_sb[:, 2 * jp:2 * jp + 2, s2],
                               start=(jp == 0), stop=False)
                        nc.tensor.matmul(ps[:, c2 * 256:(c2 + 1) * 256],
                                         lhsT=mu_bc[0:1, st * P:(st + 1) * P],
                                         rhs=nw1v_sb[0:1, s2],
                                         start=False, stop=True)
                    with nc.allow_low_precision(reason="v to fp8"):
                        nc.vector.scalar_tensor_tensor(
                            out=v8w[:, st // 2, st % 2, 8 * eh:8 * (eh + 1), 0:DH],
                            in0=ps.rearrange("p (h d) -> p h d", d=DH),
                            scalar=b32_sb[:, st:st + 1],
                            in1=binv_bc[:, sl].rearrange("p (h d) -> p h d", d=DH),
                            op0=OP.mult, op1=OP.add)

            def alloc_ex(hp):
                return expool.tile([P, 2, NS, S], fp8, tag="ex", bufs=2,
                                   name=f"ex{hp}")

            def emit_scores(hp):
                # scores^T[t, s] per head pair + exp (ACT / DVE-copy+Pool)
                ex_t = alloc_ex(hp)
                for tt in range(NS):
                    for idx in range(2):
                        h = 2 * hp + idx
                        base = idx * DH
                        ps = mps.tile([P, S], f32, tag="sc", bufs=2,
                                      name=f"sc{hp}_{tt}_{idx}")
                        for sh in range(2):
                            sl = slice(sh * 512, (sh + 1) * 512)
                            nc.tensor.matmul(
                                ps[:, sl],
                                lhsT=qkT[base:base + DH, 8 + hp, tt * P:(tt + 1) * P],
                                rhs=qkT[base:base + DH, hp, sl],
                                start=True, stop=True, tile_position=(base, 0))
                        with nc.allow_low_precision(reason="exp to fp8"):
                            if tt in POOL_TT:
                                sst = sidep.tile([P, S], f32, tag="sst",
                                                 bufs=2, name=f"sst{hp}_{idx}")
                                nc.vector.tensor_scalar_add(
                                    sst, ps, bKr_sb[:, h, tt:tt + 1])
                                nc.gpsimd.tensor_tensor(
                                    out=ex_t[:, idx, tt, :], in0=ebase,
                                    in1=sst, op=OP.pow)
                            else:
                                nc.scalar.activation(
                                    out=ex_t[:, idx, tt, :], in_=ps,
                                    func=FT.Exp, scale=EXP_SCALE,
                                    bias=bK_sb[:, h, tt:tt + 1])
                return ex_t

            def emit_pvden(hp, ex_t):
                # PV + fused den ([v | ones/CS] stationary), then normalize
                for sh in range(2):
                    for idx in range(2):
                        h = 2 * hp + idx
                        ps = mps.tile([P, 512], f32, tag="mm", bufs=3,
                                      name=f"pv{hp}_{sh}_{idx}")
                        for c2 in range(2):
                            s2 = slice(sh * 512 + c2 * 256,
                                       sh * 512 + (c2 + 1) * 256)
                            for tp in range(NS // 2):
                                dr(ps[:, c2 * 256:(c2 + 1) * 256],
                                   v8w[:, tp, :, h, :],
                                   ex_t[:, idx, 2 * tp:2 * tp + 2, s2],
                                   start=(tp == 0),
                                   stop=(tp == NS // 2 - 1))
                        sl = slice(sh * 512, (sh + 1) * 512)
                        rden = sidep.tile([DH, 512], bf16, tag="rd", bufs=4,
                                          name=f"rd{hp}_{sh}_{idx}")
                        with nc.allow_low_precision(reason="denom in bf16"):
                            nc.vector.reciprocal(out=rden, in_=ps[DH:P, :])
                            nc.vector.tensor_tensor(
                                out=ctxT8[idx * DH:(idx + 1) * DH, hp, sl],
                                in0=ps[0:DH, :], in1=rden, op=OP.mult)

            def emit_outproj_a(sts):
                # heads 0-7 partial -> stage (residual folded in)
                for st in sts:
                    s0 = st * P
                    for eh in range(2):
                        ps = mps.tile([P, 512], f32, tag="mm", bufs=3,
                                      name=f"opa{st}_{eh}")
                        sl = slice(eh * 512, (eh + 1) * 512)
                        for c2 in range(2):
                            s2 = slice(eh * 512 + c2 * 256,
                                       eh * 512 + (c2 + 1) * 256)
                            for q in range(2):
                                dr(ps[:, c2 * 256:(c2 + 1) * 256],
                                   ctxT8[:, 2 * q:2 * q + 2, s0:s0 + P],
                                   wout_sb[:, 2 * q:2 * q + 2, s2],
                                   start=(q == 0), stop=(q == 1))
                        with nc.allow_low_precision(reason="stage in bf16"):
                            nc.vector.scalar_tensor_tensor(
                                out=stage_sb[:, st, sl], in0=ps,
                                scalar=OUT_SCALE, in1=resid_sb[:, st, sl],
                                op0=OP.mult, op1=OP.add)

            def emit_outproj_b():
                for st in range(NS):
                    s0 = st * P
                    for eh in range(2):
                        ps = mps.tile([P, 512], f32, tag="mm", bufs=3,
                                      name=f"opb{st}_{eh}")
                        sl = slice(eh * 512, (eh + 1) * 512)
                        for c2 in range(2):
                            s2 = slice(eh * 512 + c2 * 256,
                                       eh * 512 + (c2 + 1) * 256)
                            for q in range(2, 4):
                                dr(ps[:, c2 * 256:(c2 + 1) * 256],
                                   ctxT8[:, 2 * q:2 * q + 2, s0:s0 + P],
                                   wout_sb[:, 2 * q:2 * q + 2, s2],
                                   start=(q == 2), stop=(q == 3))
                        ob = sidep.tile([P, 512], bf16, tag="ob", bufs=3,
                                        name=f"ob{st}_{eh}")
                        with nc.allow_low_precision(reason="out in bf16"):
                            nc.vector.scalar_tensor_tensor(
                                out=ob, in0=ps, scalar=OUT_SCALE,
                                in1=stage_sb[:, st, sl],
                                op0=OP.mult, op1=OP.add)
                        nc.sync.dma_start(out=out[s0:s0 + P, sl], in_=ob)

            # ---- interleaved emission ----
            for et in (0, 8):
                emit_qk_unit(et)
            emit_bk(8)
            for et in (1, 9):
                emit_qk_unit(et)
            emit_bk(9)
            ex_prev = emit_scores(0)
            for p_ in range(2, _ND):
                emit_qk_unit(p_)
                emit_qk_unit(8 + p_)
                emit_bk(8 + p_)
            for st in range(NS):
                emit_v_unit(st)
            ex_cur = emit_scores(1)
            emit_pvden(0, ex_prev)
            ex_prev = ex_cur
            for hp in range(2, H // 2):
                ex_cur = emit_scores(hp)
                emit_pvden(hp - 1, ex_prev)
                ex_prev = ex_cur
                if hp >= 4:
                    emit_outproj_a(range(2 * (hp - 4), 2 * (hp - 3)))
            nc.sync.dma_start(out=dbg_ex, in_=ex_prev)
            emit_pvden(H // 2 - 1, ex_prev)
            emit_outproj_b()
            nc.sync.dma_start(out=dbg_qkT, in_=qkT)
            nc.sync.dma_start(out=dbg_bbc, in_=b_bc)


def build_nc():
    import concourse.bacc as bacc
    import concourse.tile as tile
    from concourse import mybir

    f32 = mybir.dt.float32
    bf16 = mybir.dt.bfloat16
    fp8 = mybir.dt.float8e4

    nc = bacc.Bacc("TRN2", target_bir_lowering=False, debug=False)
    aps = {
        "xt": nc.dram_tensor("xt", [D, S], bf16, kind="ExternalInput").ap(),
        "x8": nc.dram_tensor("x8", [D, S], fp8, kind="ExternalInput").ap(),
        "resid": nc.dram_tensor("resid", [S, D], bf16, kind="ExternalInput").ap(),
        "wqkt": nc.dram_tensor("wqkt", [D, 2 * D], fp8, kind="ExternalInput").ap(),
        "wvt": nc.dram_tensor("wvt", [D, D], fp8, kind="ExternalInput").ap(),
        "woutt": nc.dram_tensor("woutt", [D, D], fp8, kind="ExternalInput").ap(),
        "negw1qk": nc.dram_tensor("negw1qk", [1, 2 * D], bf16, kind="ExternalInput").ap(),
        "negw1v": nc.dram_tensor("negw1v", [1, D], bf16, kind="ExternalInput").ap(),
        "bq32": nc.dram_tensor("bq32", [P, H], fp8, kind="ExternalInput").ap(),
        "binv": nc.dram_tensor("binv", [D], f32, kind="ExternalInput").ap(),
        "out": nc.dram_tensor("out", [S, D], bf16, kind="ExternalOutput").ap(),
        "dbg_qkT": nc.dram_tensor("dbg_qkT", [P, 16, S], mybir.dt.float8e4, kind="ExternalOutput").ap(),
        "dbg_bbc": nc.dram_tensor("dbg_bbc", [P, S], bf16, kind="ExternalOutput").ap(),
        "dbg_ch": nc.dram_tensor("dbg_ch", [P, 48], mybir.dt.float32, kind="ExternalOutput").ap(),
        "dbg_ex": nc.dram_tensor("dbg_ex", [P, 2, 8, S], mybir.dt.float8e4, kind="ExternalOutput").ap(),
    }
    with tile.TileContext(nc) as tc:
        _emit(tc, aps)
    nc.compile()
    return nc


def prep_inputs(x, ln_gamma, ln_beta, in_proj_w, in_proj_b, out_proj_w, out_proj_b,
                n_cores=N_CORES):
    bf = ml_dtypes.bfloat16
    f8 = ml_dtypes.float8_e4m3
    win = np.asarray(in_proj_w, np.float32)
    g = np.asarray(ln_gamma, np.float32)
    bt = np.asarray(ln_beta, np.float32)
    bin_ = np.asarray(in_proj_b, np.float32)
    wing = win * g[None, :]          # gamma folded into in-proj columns
    binf = bin_ + win @ bt           # beta folded into the in-proj biases
    wqkt8 = np.ascontiguousarray((wing[:2 * D] * WS).T).astype(f8)
    wvt8 = np.ascontiguousarray((wing[2 * D:] * WS).T).astype(f8)
    negw1qk = -wqkt8.astype(np.float32).sum(axis=0, keepdims=True)
    negw1v = -wvt8.astype(np.float32).sum(axis=0, keepdims=True)
    # bq (q-bias) stacked per head parity: [64*(h%2)+dh, h] = WS*binf[h*64+dh]
    bq32 = np.zeros((P, H), np.float32)
    for h in range(H):
        bq32[(h % 2) * DH:(h % 2) * DH + DH, h] = WS * binf[h * DH:(h + 1) * DH]
    shared = {
        "wqkt": wqkt8,
        "wvt": wvt8,
        "woutt": np.ascontiguousarray(np.asarray(out_proj_w, np.float32).T * WS).astype(f8),
        "negw1qk": negw1qk.astype(bf),
        "negw1v": negw1v.astype(bf),
        "bq32": bq32.astype(f8),
        "binv": np.ascontiguousarray(binf[2 * D:], np.float32),
    }
    bout = np.asarray(out_proj_b, np.float32)
    in_maps = []
    for i in range(n_cores):
        xi = np.asarray(x[i], np.float32)
        m = dict(shared)
        xit = np.ascontiguousarray(xi.T)
        m["xt"] = xit.astype(bf)
        m["x8"] = xit.astype(f8)
        m["resid"] = np.ascontiguousarray(xi + bout).astype(bf)
        in_maps.append(m)
    return in_maps


def kernel(x, ln_gamma, ln_beta, in_proj_w, in_proj_b, out_proj_w, out_proj_b):
    global LAST_RESULTS
    from concourse import bass_utils

    if "nc" not in _NC_CACHE:
        _NC_CACHE["nc"] = build_nc()
    nc = _NC_CACHE["nc"]

    in_maps = prep_inputs(x, ln_gamma, ln_beta, in_proj_w, in_proj_b,
                          out_proj_w, out_proj_b)
    res = bass_utils.run_bass_kernel_spmd(nc, in_maps, core_ids=list(range(N_CORES)))
    LAST_RESULTS = res
    out = np.stack([r["out"] for r in res.results], axis=0)
    return np.ascontiguousarray(out, dtype=np.float32)


# revision 25
# speedup vs baseline: 1.0716x; 1.0716x over previous
"""Self-contained Trainium2 Bass kernel: pre-LN multi-head attention block.

Computes, for x [B=8, S=1024, D=1024] (fp32) and packed attention weights:
    out = x + out_proj(MHA(LayerNorm(x)))
matching torch nn.MultiheadAttention's explicit (non-flash) path with 16 heads.

Sharding: data-parallel over batch - core i handles batch element i; no
collectives, outputs are concatenated on the host.

Per-core strategy (v2):
  - x arrives twice: bf16 x^T for LN stats, fp8 x^T (plane layout) for the
    QKV DoubleRow matmuls. LayerNorm is folded ALGEBRAICALLY into QKV:
      q = rstd[s]*(x@W) - mu[s]*colsum(W) (+ gamma pre-folded into W),
    so the normalized activations are never materialized. The rank-1
    mu*w1 correction is one extra 1-row matmul pass per output tile; the
    rstd multiply rides the mandatory PSUM->SBUF drain.
  - LN stats are 1-column-moving matmuls (cost ~1 cycle each in the PE
    cost model), chained through a PE transpose and broadcast DMAs.
  - q/k biases (LN beta fold) are folded into the softmax: the bq.K term
    becomes the exp's per-partition bias (tiny 1-col matmuls); the bk.q
    term is constant per softmax column and cancels between num/denom.
  - softmax exp splits across TWO engines: most tiles on ACT
    (Exp, scale=2^-13), the rest via a DVE psum->sbuf copy + gpsimd
    tensor_tensor pow(e^(2^-13), scores) - the Pool engine computes exp
    at 0.6 efficiency but runs fully parallel to ACT.
  - PV uses a wide [v | ones/16] stationary so the softmax denominator
    accumulates in psum partitions 64-127 of the same matmul (denominator
    matmuls cost nothing extra); ctx is normalized at x16 scale to stay
    in fp8-normal range, with the v-bias folded via ctx = PV/den + bv.
  - out-proj is split in two head-halves so the final drain+DMA overlaps
    the last heads' attention.
"""

import numpy as np
import ml_dtypes

P = 128
D = 1024
H = 16
DH = 64
B = 8
S = 1024
LN_EPS = 1e-5
N_CORES = 8

_ND = D // P   # d tiles (8)
NS = S // P    # s tiles (8)
WS = 32.0      # fp8 weight pre-scale (power of two)
EXP_SCALE = 0.125 / 1024.0      # 2^-13: logits = psum * 2^-13
CS = 16.0                       # ctx fp8 scale (den ones = 1/CS)
OUT_SCALE = 1.0 / (CS * WS)     # 1/512
EBASE = float(np.exp(2.0 ** -13))
EXP_BIAS = -3.0   # cancels in softmax; keeps fp8 exp finite

# exp tiles routed to the Pool engine: (tt indices per (hp, idx) head)
POOL_TT = (7,)

LAST_RESULTS = None
_NC_CACHE = {}


def _emit(tc, aps):
    from concourse import mybir
    from concourse.masks import make_identity

    nc = tc.nc
    f32 = mybir.dt.float32
    bf16 = mybir.dt.bfloat16
    fp8 = mybir.dt.float8e4
    FT = mybir.ActivationFunctionType
    OP = mybir.AluOpType
    DR = mybir.MatmulPerfMode.DoubleRow

    x8d, resid, wqkt, wvt, woutt, negw1qk, negw1v, bq32, binv, out = (
        aps["x8"], aps["resid"], aps["wqkt"], aps["wvt"],
        aps["woutt"], aps["negw1qk"], aps["negw1v"], aps["bq32"],
        aps["binv"], aps["out"],
    )

    with tc.tile_pool(name="consts", bufs=1) as consts, \
         tc.tile_pool(name="acts", bufs=1) as acts, \
         tc.tile_pool(name="wpool", bufs=1) as wpool:

        # ---------- constants ----------
        ident = consts.tile([P, P], bf16)
        make_identity(nc, ident[:])
        ones_mat = consts.tile([P, P], bf16)
        nc.vector.memset(ones_mat, 1.0)
        ones1 = consts.tile([P, 1], bf16)
        nc.vector.memset(ones1, 1.0)
        ones1f8 = consts.tile([P, 1], fp8)
        nc.vector.memset(ones1f8, 1.0)
        # row-selector stationaries: sel[p, r, f] = (p == r)
        sel = consts.tile([16, 16, P], bf16)
        nc.gpsimd.memset(sel, 0.0)
        nc.gpsimd.affine_select(
            out=sel, in_=sel, compare_op=mybir.AluOpType.not_equal,
            fill=1.0, base=0,
            # keep 0 where (p - r) != 0, fill 1.0 on p == r
            pattern=[[-1, 16], [0, P]], channel_multiplier=1)
        eps_sb = consts.tile([P, 1], f32)
        nc.vector.memset(eps_sb, LN_EPS)
        ebase = consts.tile([P, S], f32)
        nc.vector.memset(ebase, EBASE)
        binv_bc = consts.tile([P, D], f32)
        nw1qk_sb = consts.tile([1, 2 * D], bf16)
        nw1v_sb = consts.tile([1, D], bf16)
        bq32_sb = consts.tile([P, H], fp8)

        # ---------- persistent activations ----------
        x8_sb = acts.tile([P, _ND, S], fp8)      # x^T fp8 (QKV + LN stats)
        qkT = acts.tile([P, 2 * _ND, S], fp8)    # q tiles 0..7, k tiles 8..15
        # V wide stationary: [t%128, ttpair, plane, h, 64 v | 64 ones/CS]
        v8w = acts.tile([P, NS // 2, 2, H, P], fp8)
        ctxT8 = acts.tile([P, _ND, S], fp8)      # x16-scaled normalized ctx^T
        resid_sb = acts.tile([P, NS, D], bf16)   # x + out_proj_b, natural
        stage_sb = acts.tile([P, NS, D], bf16)   # heads 0-7 partial + resid
        b_bc = acts.tile([P, S], bf16)           # rstd, partition-replicated
        mu_bc = acts.tile([P, S], bf16)          # mu, partition-replicated
        bK_sb = acts.tile([P, H, NS], f32)       # exp bias: 2^-13*(bq.K)[t]
        bKr_sb = acts.tile([P, H, NS], f32)      # same, raw psum scale
        b32_sb = acts.tile([P, NS], f32)         # rstd/WS per t (V drain)

        # ---------- weights (fp8, pre-scaled by WS on host) ----------
        wqk_sb = wpool.tile([P, _ND, 2 * D], fp8)
        wv_sb = wpool.tile([P, _ND, D], fp8)
        wout_sb = wpool.tile([P, _ND, D], fp8)

        # memset the ones-planes of the V wide stationary (den columns)
        nc.gpsimd.memset(v8w[:, :, :, :, DH:P], 1.0 / CS)

        # ================= Phase 1: LayerNorm stats =================
        with tc.tile_pool(name="lnps", bufs=1, space="PSUM") as lnps, \
             tc.tile_pool(name="lntmp", bufs=1) as lntmp:
            stats_ps = lnps.tile([P, 2 * NS], f32, tag="stats")
            tps = lnps.tile([16, P], bf16, tag="tps")
            rep_ps = lnps.tile([P, 2 * S], f32, tag="rep")
            warm_ps = lnps.tile([P, P], f32, tag="warm")
            for _ in range(24):
                nc.tensor.matmul(warm_ps, lhsT=ones_mat, rhs=ones_mat,
                                 start=True, stop=True)
            NCK = 512
            x8_r = x8d.rearrange("(a p) s -> p a s", p=P)
            for c in range(2):
                sl = slice(c * NCK, (c + 1) * NCK)
                for j in range(_ND):
                    nc.sync.dma_start(out=x8_sb[:, j, sl], in_=x8_r[:, j, sl])
            wqkt_r2 = wqkt.rearrange("(a p) e -> p a e", p=P)
            nc.sync.dma_start(out=wqk_sb[:, :, 0:2 * P],
                              in_=wqkt_r2[:, :, 0:2 * P])
            nc.sync.dma_start(out=wqk_sb[:, :, 8 * P:10 * P],
                              in_=wqkt_r2[:, :, 8 * P:10 * P])
            nc.sync.dma_start(out=wqk_sb[:, :, 2 * P:8 * P],
                              in_=wqkt_r2[:, :, 2 * P:8 * P])
            nc.sync.dma_start(out=wqk_sb[:, :, 10 * P:16 * P],
                              in_=wqkt_r2[:, :, 10 * P:16 * P])
            nc.sync.dma_start(out=wv_sb,
                              in_=wvt.rearrange("(a p) e -> p a e", p=P))
            nc.sync.dma_start(out=wout_sb,
                              in_=woutt.rearrange("(a p) e -> p a e", p=P))
            nc.sync.dma_start(out=resid_sb,
                              in_=resid.rearrange("(st p) e -> p st e", p=P))
            nc.gpsimd.dma_start(out=binv_bc,
                                in_=binv[None, :].to_broadcast((P, D)))
            nc.gpsimd.dma_start(out=nw1qk_sb, in_=negw1qk)
            nc.gpsimd.dma_start(out=nw1v_sb, in_=negw1v)
            nc.gpsimd.dma_start(out=bq32_sb, in_=bq32)

            # squares (DVE, bf16 2x) + 1-col stats matmuls; accumulation
            # groups must be strictly sequential (interleaved start/stop
            # groups in one psum bank corrupt each other)
            for c in range(2):
                sqs = []
                with nc.allow_low_precision(reason="x^2 for LN stats"):
                    for j in range(_ND):
                        sl = slice(c * NCK, (c + 1) * NCK)
                        sq = lntmp.tile([P, NCK], bf16, tag="sq", bufs=16,
                                        name=f"sq{c}_{j}")
                        nc.vector.tensor_tensor(out=sq, in0=x8_sb[:, j, sl],
                                                in1=x8_sb[:, j, sl],
                                                op=OP.mult)
                        sqs.append(sq)
                for st in range(c * 4, c * 4 + 4):
                    o = st * P - c * NCK
                    for j in range(_ND):
                        nc.tensor.matmul(stats_ps[:, st:st + 1],
                                         lhsT=x8_sb[:, j, st * P:(st + 1) * P],
                                         rhs=ones1f8,
                                         start=(j == 0), stop=(j == _ND - 1))
                    for j in range(_ND):
                        nc.tensor.matmul(stats_ps[:, NS + st:NS + st + 1],
                                         lhsT=sqs[j][:, o:o + P], rhs=ones1,
                                         start=(j == 0), stop=(j == _ND - 1))

            # chain on [128, 8] tiles: partition = s%128, col = s-tile
            ch = lntmp.tile([P, 6 * NS], f32, tag="ch")
            mu = ch[:, 0:NS]
            var = ch[:, NS:2 * NS]
            std = ch[:, 2 * NS:3 * NS]
            b8f = ch[:, 3 * NS:4 * NS]
            nc.vector.tensor_scalar_mul(mu, stats_ps[:, 0:NS], 1.0 / D)
            with nc.allow_low_precision(reason="LN chain"):
                musq = ch[:, 4 * NS:5 * NS]
                nc.vector.tensor_tensor(out=musq, in0=mu, in1=mu, op=OP.mult)
                nc.vector.scalar_tensor_tensor(
                    out=var, in0=stats_ps[:, NS:2 * NS], scalar=1.0 / D,
                    in1=musq, op0=OP.mult, op1=OP.subtract)
            nc.scalar.activation(out=std, in_=var, func=FT.Sqrt, bias=eps_sb)
            nc.vector.reciprocal(out=b8f, in_=std)
            nc.vector.tensor_scalar_mul(b32_sb, b8f, 1.0 / WS)
            bmu = lntmp.tile([P, 16], bf16, tag="bmu")
            with nc.allow_low_precision(reason="LN stats to bf16"):
                nc.vector.tensor_copy(out=bmu[:, 0:NS], in_=b8f)
                nc.vector.tensor_copy(out=bmu[:, NS:16], in_=mu)
            nc.tensor.transpose(out=tps, in_=bmu, identity=ident)
            tsb = lntmp.tile([16, P], bf16, tag="tsb")
            nc.vector.tensor_copy(out=tsb, in_=tps)
            # replicate b and mu across partitions via selector matmuls
            for st in range(NS):
                nc.tensor.matmul(rep_ps[:, st * P:(st + 1) * P],
                                 lhsT=sel[:, st, :], rhs=tsb,
                                 start=True, stop=True)
                nc.tensor.matmul(rep_ps[:, S + st * P:S + (st + 1) * P],
                                 lhsT=sel[:, NS + st, :], rhs=tsb,
                                 start=True, stop=True)
            with nc.allow_low_precision(reason="LN bcast to bf16"):
                nc.vector.tensor_copy(out=b_bc, in_=rep_ps[:, 0:S])
                nc.vector.tensor_copy(out=mu_bc, in_=rep_ps[:, S:2 * S])

        # ============ Phases 2-4: projections + attention + out-proj ========
        with tc.tile_pool(name="expool", bufs=1) as expool, \
             tc.tile_pool(name="sidep", bufs=1) as sidep, \
             tc.tile_pool(name="mps", bufs=1, space="PSUM") as mps:

            def dr(ps_out, lhsT, rhs, start, stop):
                nc.tensor.matmul(ps_out, lhsT=lhsT, rhs=rhs, start=start,
                                 stop=stop, perf_mode=DR)

            def emit_qk_half(et, half):
                # one q|k e-tile s-half [128e, 512]: 4 DR + mu*w1 correction
                e0 = et * P
                if True:
                    ps = mps.tile([P, 512], f32, tag="mm", bufs=3,
                                  name=f"qk{et}_{half}")
                    sl = slice(half * 512, (half + 1) * 512)
                    for c2 in range(2):
                        s2 = slice(half * 512 + c2 * 256,
                                   half * 512 + (c2 + 1) * 256)
                        for jp in range(_ND // 2):
                            dr(ps[:, c2 * 256:(c2 + 1) * 256],
                               wqk_sb[:, 2 * jp:2 * jp + 2, e0:e0 + P],
                               x8_sb[:, 2 * jp:2 * jp + 2, s2],
                               start=(jp == 0), stop=False)
                        nc.tensor.matmul(ps[:, c2 * 256:(c2 + 1) * 256],
                                         lhsT=nw1qk_sb[0:1, e0:e0 + P],
                                         rhs=mu_bc[0:1, s2],
                                         start=False, stop=True)
                    with nc.allow_low_precision(reason="qk to bf16"):
                        nc.vector.tensor_tensor(out=qkT[:, et, sl], in0=ps,
                                                in1=b_bc[:, sl], op=OP.mult)

            def emit_bk(et):
                # exp-bias fold: bK[t, h] = 2^-13 * sum_dh bq[dh] k[dh, t]
                # (k e-tile et covers heads 2(et-8), 2(et-8)+1)
                bps = mps.tile([P, 2 * NS], f32, tag="bk", bufs=1,
                               name=f"bk{et}")
                for idx in range(2):
                    h = 2 * (et - 8) + idx
                    base = idx * DH
                    for tt in range(NS):
                        nc.tensor.matmul(
                            bps[:, idx * NS + tt:idx * NS + tt + 1],
                            lhsT=qkT[base:base + DH, et, tt * P:(tt + 1) * P],
                            rhs=bq32_sb[base:base + DH, h:h + 1],
                            start=True, stop=True, tile_position=(base, 0))
                for idx in range(2):
                    h = 2 * (et - 8) + idx
                    sl = slice(idx * NS, (idx + 1) * NS)
                    nc.vector.tensor_scalar(out=bK_sb[:, h, :], in0=bps[:, sl],
                                            scalar1=EXP_SCALE, scalar2=EXP_BIAS,
                                            op0=OP.mult, op1=OP.add)
                    nc.vector.tensor_scalar(out=bKr_sb[:, h, :], in0=bps[:, sl],
                                            scalar1=1.0,
                                            scalar2=EXP_BIAS * 8192.0,
                                            op0=OP.mult, op1=OP.add)

            def emit_v_unit(st, eh):
                # V natural [t-tile, e'=(h,dh)] per e'-half: 4 DR + mu*w1v fix
                t0 = st * P
                if True:
                    ps = mps.tile([P, 512], f32, tag="mm", bufs=3,
                                  name=f"v{st}_{eh}")
                    sl = slice(eh * 512, (eh + 1) * 512)
                    for c2 in range(2):
                        s2 = slice(eh * 512 + c2 * 256,
                                   eh * 512 + (c2 + 1) * 256)
                        for jp in range(_ND // 2):
                            dr(ps[:, c2 * 256:(c2 + 1) * 256],
                               x8_sb[:, 2 * jp:2 * jp + 2, t0:t0 + P],
                               wv_sb[:, 2 * jp:2 * jp + 2, s2],
                               start=(jp == 0), stop=False)
                        nc.tensor.matmul(ps[:, c2 * 256:(c2 + 1) * 256],
                                         lhsT=mu_bc[0:1, st * P:(st + 1) * P],
                                         rhs=nw1v_sb[0:1, s2],
                                         start=False, stop=True)
                    with nc.allow_low_precision(reason="v to fp8"):
                        nc.vector.scalar_tensor_tensor(
                            out=v8w[:, st // 2, st % 2, 8 * eh:8 * (eh + 1), 0:DH],
                            in0=ps.rearrange("p (h d) -> p h d", d=DH),
                            scalar=b32_sb[:, st:st + 1],
                            in1=binv_bc[:, sl].rearrange("p (h d) -> p h d", d=DH),
                            op0=OP.mult, op1=OP.add)

            def alloc_ex(hp):
                return expool.tile([P, 2, NS, S], fp8, tag="ex", bufs=3,
                                   name=f"ex{hp}")

            def emit_score_unit(hp, ex_t, tt, idx):
                # one scores^T tile [t-tile, S] + exp (ACT / DVE-copy+Pool)
                if True:
                    if True:
                        h = 2 * hp + idx
                        base = idx * DH
                        ps = mps.tile([P, S], f32, tag="sc", bufs=2,
                                      name=f"sc{hp}_{tt}_{idx}")
                        for sh in range(2):
                            sl = slice(sh * 512, (sh + 1) * 512)
                            nc.tensor.matmul(
                                ps[:, sl],
                                lhsT=qkT[base:base + DH, 8 + hp, tt * P:(tt + 1) * P],
                                rhs=qkT[base:base + DH, hp, sl],
                                start=True, stop=True, tile_position=(base, 0))
                        with nc.allow_low_precision(reason="exp to fp8"):
                            if tt in POOL_TT:
                                sst = sidep.tile([P, S], f32, tag="sst",
                                                 bufs=2, name=f"sst{hp}_{idx}")
                                nc.vector.tensor_scalar_add(
                                    sst, ps, bKr_sb[:, h, tt:tt + 1])
                                nc.gpsimd.tensor_tensor(
                                    out=ex_t[:, idx, tt, :], in0=ebase,
                                    in1=sst, op=OP.pow)
                            else:
                                nc.scalar.activation(
                                    out=ex_t[:, idx, tt, :], in_=ps,
                                    func=FT.Exp, scale=EXP_SCALE,
                                    bias=bK_sb[:, h, tt:tt + 1])

            def emit_pv_unit(hp, ex_t, sh, idx):
                # PV + fused den ([v | ones/CS] stationary), then normalize
                if True:
                    if True:
                        h = 2 * hp + idx
                        ps = mps.tile([P, 512], f32, tag="mm", bufs=3,
                                      name=f"pv{hp}_{sh}_{idx}")
                        for c2 in range(2):
                            s2 = slice(sh * 512 + c2 * 256,
                                       sh * 512 + (c2 + 1) * 256)
                            for tp in range(NS // 2):
                                dr(ps[:, c2 * 256:(c2 + 1) * 256],
                                   v8w[:, tp, :, h, :],
                                   ex_t[:, idx, 2 * tp:2 * tp + 2, s2],
                                   start=(tp == 0),
                                   stop=(tp == NS // 2 - 1))
                        sl = slice(sh * 512, (sh + 1) * 512)
                        rden = sidep.tile([DH, 512], bf16, tag="rd", bufs=4,
                                          name=f"rd{hp}_{sh}_{idx}")
                        with nc.allow_low_precision(reason="denom in bf16"):
                            nc.vector.reciprocal(out=rden, in_=ps[DH:P, :])
                            nc.vector.tensor_tensor(
                                out=ctxT8[idx * DH:(idx + 1) * DH, hp, sl],
                                in0=ps[0:DH, :], in1=rden, op=OP.mult)

            def emit_opa_unit(st, eh):
                # heads 0-7 partial -> stage (residual folded in)
                if True:
                    s0 = st * P
                    if True:
                        ps = mps.tile([P, 512], f32, tag="mm", bufs=3,
                                      name=f"opa{st}_{eh}")
                        sl = slice(eh * 512, (eh + 1) * 512)
                        for c2 in range(2):
                            s2 = slice(eh * 512 + c2 * 256,
                                       eh * 512 + (c2 + 1) * 256)
                            for q in range(2):
                                dr(ps[:, c2 * 256:(c2 + 1) * 256],
                                   ctxT8[:, 2 * q:2 * q + 2, s0:s0 + P],
                                   wout_sb[:, 2 * q:2 * q + 2, s2],
                                   start=(q == 0), stop=(q == 1))
                        with nc.allow_low_precision(reason="stage in bf16"):
                            nc.vector.scalar_tensor_tensor(
                                out=stage_sb[:, st, sl], in0=ps,
                                scalar=OUT_SCALE, in1=resid_sb[:, st, sl],
                                op0=OP.mult, op1=OP.add)

            def emit_opb_unit(st, eh):
                if True:
                    s0 = st * P
                    if True:
                        ps = mps.tile([P, 512], f32, tag="mm", bufs=3,
                                      name=f"opb{st}_{eh}")
                        sl = slice(eh * 512, (eh + 1) * 512)
                        for c2 in range(2):
                            s2 = slice(eh * 512 + c2 * 256,
                                       eh * 512 + (c2 + 1) * 256)
                            for q in range(2, 4):
                                dr(ps[:, c2 * 256:(c2 + 1) * 256],
                                   ctxT8[:, 2 * q:2 * q + 2, s0:s0 + P],
                                   wout_sb[:, 2 * q:2 * q + 2, s2],
                                   start=(q == 2), stop=(q == 3))
                        ob = sidep.tile([P, 512], bf16, tag="ob", bufs=3,
                                        name=f"ob{st}_{eh}")
                        with nc.allow_low_precision(reason="out in bf16"):
                            nc.vector.scalar_tensor_tensor(
                                out=ob, in0=ps, scalar=OUT_SCALE,
                                in1=stage_sb[:, st, sl],
                                op0=OP.mult, op1=OP.add)
                        nc.sync.dma_start(out=out[s0:s0 + P, sl], in_=ob)

            # ---- interleaved emission: zip score units with filler work
            # so no engine queue drains between phases ----
            def zipper(primary, filler):
                n, m = len(primary), len(filler)
                fi = 0
                for i, p_fn in enumerate(primary):
                    p_fn()
                    want = ((i + 1) * m) // n
                    while fi < want:
                        filler[fi]()
                        fi += 1
                while fi < m:
                    filler[fi]()
                    fi += 1

            def sc_units(hp, ex_t):
                return [
                    (lambda tt=tt, idx=idx: emit_score_unit(hp, ex_t, tt, idx))
                    for tt in range(NS) for idx in range(2)
                ]

            def pv_units(hp, ex_t):
                return [
                    (lambda sh=sh, idx=idx: emit_pv_unit(hp, ex_t, sh, idx))
                    for sh in range(2) for idx in range(2)
                ]

            # prologue: first q/k e-tiles + their exp-bias folds
            for et in (0, 8):
                for half in range(2):
                    emit_qk_half(et, half)
            emit_bk(8)
            for et in (1, 9):
                for half in range(2):
                    emit_qk_half(et, half)
            emit_bk(9)

            # remaining projection work, consumed as filler during scores
            proj = []
            for p_ in range(2, _ND):
                for half in range(2):
                    proj.append(lambda et=p_, h_=half: emit_qk_half(et, h_))
                for half in range(2):
                    proj.append(lambda et=8 + p_, h_=half: emit_qk_half(et, h_))
                proj.append(lambda et=8 + p_: emit_bk(et))
            for st in range(NS):
                for eh in range(2):
                    proj.append(lambda s_=st, e_=eh: emit_v_unit(s_, e_))

            ex_prev = alloc_ex(0)
            zipper(sc_units(0, ex_prev), proj[:18])
            ex_cur = alloc_ex(1)
            zipper(sc_units(1, ex_cur), proj[18:])
            for hp in range(2, H // 2):
                ex_nxt = alloc_ex(hp)
                filler = pv_units(hp - 2, ex_prev)
                if hp >= 5:
                    st0 = 2 * (hp - 5)
                    filler += [
                        (lambda s_=st, e_=eh: emit_opa_unit(s_, e_))
                        for st in (st0, st0 + 1) for eh in range(2)
                    ]
                zipper(sc_units(hp, ex_nxt), filler)
                ex_prev, ex_cur = ex_cur, ex_nxt
            tail = pv_units(H // 2 - 2, ex_prev) + pv_units(H // 2 - 1, ex_cur)
            tail += [
                (lambda s_=st, e_=eh: emit_opa_unit(s_, e_))
                for st in (6, 7) for eh in range(2)
            ]
            opb = [
                (lambda s_=st, e_=eh: emit_opb_unit(s_, e_))
                for st in range(NS) for eh in range(2)
            ]
            for fn in tail:
                fn()
            for fn in opb:
                fn()


def build_nc():
    import concourse.bacc as bacc
    import concourse.tile as tile
    from concourse import mybir

    f32 = mybir.dt.float32
    bf16 = mybir.dt.bfloat16
    fp8 = mybir.dt.float8e4

    nc = bacc.Bacc("TRN2", target_bir_lowering=False, debug=False)
    aps = {
        "x8": nc.dram_tensor("x8", [D, S], fp8, kind="ExternalInput").ap(),
        "resid": nc.dram_tensor("resid", [S, D], bf16, kind="ExternalInput").ap(),
        "wqkt": nc.dram_tensor("wqkt", [D, 2 * D], fp8, kind="ExternalInput").ap(),
        "wvt": nc.dram_tensor("wvt", [D, D], fp8, kind="ExternalInput").ap(),
        "woutt": nc.dram_tensor("woutt", [D, D], fp8, kind="ExternalInput").ap(),
        "negw1qk": nc.dram_tensor("negw1qk", [1, 2 * D], bf16, kind="ExternalInput").ap(),
        "negw1v": nc.dram_tensor("negw1v", [1, D], bf16, kind="ExternalInput").ap(),
        "bq32": nc.dram_tensor("bq32", [P, H], fp8, kind="ExternalInput").ap(),
        "binv": nc.dram_tensor("binv", [D], f32, kind="ExternalInput").ap(),
        "out": nc.dram_tensor("out", [S, D], bf16, kind="ExternalOutput").ap(),
    }
    with tile.TileContext(nc) as tc:
        _emit(tc, aps)
    nc.compile()
    return nc


def prep_inputs(x, ln_gamma, ln_beta, in_proj_w, in_proj_b, out_proj_w, out_proj_b,
                n_cores=N_CORES):
    bf = ml_dtypes.bfloat16
    f8 = ml_dtypes.float8_e4m3
    win = np.asarray(in_proj_w, np.float32)
    g = np.asarray(ln_gamma, np.float32)
    bt = np.asarray(ln_beta, np.float32)
    bin_ = np.asarray(in_proj_b, np.float32)
    wing = win * g[None, :]          # gamma folded into in-proj columns
    binf = bin_ + win @ bt           # beta folded into the in-proj biases
    wqkt8 = np.ascontiguousarray((wing[:2 * D] * WS).T).astype(f8)
    wvt8 = np.ascontiguousarray((wing[2 * D:] * WS).T).astype(f8)
    negw1qk = -wqkt8.astype(np.float32).sum(axis=0, keepdims=True)
    negw1v = -wvt8.astype(np.float32).sum(axis=0, keepdims=True)
    # bq (q-bias) stacked per head parity: [64*(h%2)+dh, h] = WS*binf[h*64+dh]
    bq32 = np.zeros((P, H), np.float32)
    for h in range(H):
        bq32[(h % 2) * DH:(h % 2) * DH + DH, h] = WS * binf[h * DH:(h + 1) * DH]
    shared = {
        "wqkt": wqkt8,
        "wvt": wvt8,
        "woutt": np.ascontiguousarray(np.asarray(out_proj_w, np.float32).T * WS).astype(f8),
        "negw1qk": negw1qk.astype(bf),
        "negw1v": negw1v.astype(bf),
        "bq32": bq32.astype(f8),
        "binv": np.ascontiguousarray(binf[2 * D:], np.float32),
    }
    bout = np.asarray(out_proj_b, np.float32)
    in_maps = []
    for i in range(n_cores):
        xi = np.asarray(x[i], np.float32)
        m = dict(shared)
        xit = np.ascontiguousarray(xi.T)
        m["x8"] = xit.astype(f8)
        m["resid"] = np.ascontiguousarray(xi + bout).astype(bf)
        in_maps.append(m)
    return in_maps


def kernel(x, ln_gamma, ln_beta, in_proj_w, in_proj_b, out_proj_w, out_proj_b):
    global LAST_RESULTS
    from concourse import bass_utils

    if "nc" not in _NC_CACHE:
        _NC_CACHE["nc"] = build_nc()
    nc = _NC_CACHE["nc"]

    in_maps = prep_inputs(x, ln_gamma, ln_beta, in_proj_w, in_proj_b,
                          out_proj_w, out_proj_b)
    res = bass_utils.run_bass_kernel_spmd(nc, in_maps, core_ids=list(range(N_CORES)))
    LAST_RESULTS = res
    out = np.stack([r["out"] for r in res.results], axis=0)
    return np.ascontiguousarray(out, dtype=np.float32)
